# revision 16
# baseline (speedup 1.0000x reference)
"""DeformablePointCluster TRN2 kernel (8 NeuronCores, SPMD, gather-free).

kernel(**inputs) -> (clamped (4,1728,3) f32, final_cluster (4,1728,128,3) f32,
                     idx (4,1728,128) int32)

Sharding: core i -> batch i//2, center half i%2 (864 centers padded to 896 = 7x128).
Selection: s = 9-d2 via K=24 bf16-split matmul (abs err ~1e-4). Coord-carrying score
fields G_d = (511-jloc)*2^15 + round(p_d*32766/30) + 1 masked by (s>0); per 512-chunk
top-8 x 2 rounds -> first-16-by-index hits/chunk; coords decoded from low 15 bits
(quant step 9.2e-4). Compaction: mask-scan + local_scatter. MLP on dequantized coords.
"""
import numpy as np
import ml_dtypes
from contextlib import ExitStack

import concourse.bass as bass
import concourse.tile as tile
from concourse import bacc, mybir
from concourse.bass_utils import run_bass_kernel_spmd

dt = mybir.dt
F32, BF16, I16, I32 = dt.float32, dt.bfloat16, dt.int16, dt.int32
ALU = mybir.AluOpType
ACTF = mybir.ActivationFunctionType

B, N, GS, K, H = 4, 16384, 12, 128, 256
M = GS * GS * GS
MH = M // 2                 # 864
MC = 896                    # padded (7 tiles)
TC = MC // 128
CS = 512
NCH = N // CS
RAD2 = 9.0
MARGIN = 4.0
BN_EPS = 1e-5
KQ = 32766.0 / 30.0
DEQ = 30.0 / 32766.0
PTT = 128

_bf = ml_dtypes.bfloat16


def _split2(x):
    h = x.astype(_bf).astype(np.float32)
    l = (x - h).astype(_bf).astype(np.float32)
    return h, l


def build_kernel():
    nc = bacc.Bacc("TRN2", target_bir_lowering=False, debug=False, num_devices=8)

    i_pts = nc.dram_tensor("points", [N, 3], F32, kind="ExternalInput").ap()
    i_grid = nc.dram_tensor("grid", [MC, 3], F32, kind="ExternalInput").ap()
    i_w20 = nc.dram_tensor("w20", [20, H], BF16, kind="ExternalInput").ap()
    i_w2s = nc.dram_tensor("w2s", [128, 4, 3], BF16, kind="ExternalInput").ap()
    i_gm = nc.dram_tensor("gmask", [3, 42, 128], BF16, kind="ExternalInput").ap()

    o_cl = nc.dram_tensor("clamped_o", [MC, 3], F32, kind="ExternalOutput").ap()
    o_fc = nc.dram_tensor("cluster_o", [MC, K, 3], F32, kind="ExternalOutput").ap()
    o_ix = nc.dram_tensor("idx_o", [MC, K], I32, kind="ExternalOutput").ap()

    with tile.TileContext(nc) as tc, ExitStack() as ctx:
        g = ctx.enter_context(tc.tile_pool(name="glob", bufs=1))

        PT = g.tile([128, PTT, 3], F32)
        nc.sync.dma_start(PT[:], i_pts.rearrange("(p c) d -> p c d", p=128))
        G3 = g.tile([3, MC], F32)
        nc.sync.dma_start(G3[:], i_grid.rearrange("m d -> d m"))
        W20 = g.tile([84, H], BF16)
        for bb in (0, 32, 64):
            nc.sync.dma_start(W20[bb:bb + 20, :], i_w20)
        W2S = g.tile([128, 4, 3], BF16)
        nc.sync.dma_start(W2S[:], i_w2s)

        ONESN = g.tile([1, 2048], BF16)
        nc.vector.memset(ONESN[:], 1.0)
        GM = []
        for d in range(3):
            gm = g.tile([42, 128], BF16, name=f"GM{d}", tag=f"GM{d}")
            nc.sync.dma_start(gm[:], i_gm[d])
            GM.append(gm)
        ONESF = g.tile([128, 128], F32)
        nc.vector.memset(ONESF[:], 1.0)
        IDENT = g.tile([128, 128], F32)
        nc.gpsimd.affine_select(IDENT[:], ONESF[:], pattern=[[-1, 128]],
                                compare_op=ALU.is_equal, fill=0.0,
                                base=0, channel_multiplier=1)

        # ---------- min/max ----------
        MN = g.tile([3, 1], F32)
        MX = g.tile([3, 1], F32)
        with ExitStack() as c0:
            pp = c0.enter_context(tc.tile_pool(name="prep0", bufs=1))
            pps = c0.enter_context(tc.tile_pool(name="prep0ps", bufs=1, space="PSUM"))
            mn_p = pp.tile([128, 3], F32)
            mx_p = pp.tile([128, 3], F32)
            ptT = PT[:].rearrange("p c d -> p d c")
            nc.vector.tensor_reduce(mn_p[:], ptT, axis=mybir.AxisListType.X, op=ALU.min)
            nc.vector.tensor_reduce(mx_p[:], ptT, axis=mybir.AxisListType.X, op=ALU.max)
            mnT = pps.tile([128, 128], F32, tag="mnT")
            mxT = pps.tile([128, 128], F32, tag="mxT")
            nc.tensor.transpose(mnT[0:3, :], mn_p[:, 0:3], IDENT[:])
            nc.tensor.transpose(mxT[0:3, :], mx_p[:, 0:3], IDENT[:])
            nc.vector.tensor_reduce(MN[:], mnT[0:3, :], axis=mybir.AxisListType.X, op=ALU.min)
            nc.vector.tensor_reduce(MX[:], mxT[0:3, :], axis=mybir.AxisListType.X, op=ALU.max)

        SCL = g.tile([3, 1], F32)
        nc.vector.scalar_tensor_tensor(SCL[:], MX[:], -8.0, MN[:], op0=ALU.add, op1=ALU.subtract)
        MN4 = g.tile([3, 1], F32)
        nc.vector.tensor_scalar(MN4[:], MN[:], 4.0, None, op0=ALU.add)
        C3 = g.tile([3, MC], F32)
        nc.vector.tensor_scalar(C3[:], G3[:], SCL[:], MN4[:], op0=ALU.mult, op1=ALU.add)

        # ---------- point-side RHS rows; matmul operand base must be 0/32/64 ----------
        # rows 0-23 d2; 32-34 ramp splits; 35 ones; 36-41 q splits (x,y,z)
        RHS = g.tile([42, N], BF16)
        with ExitStack() as c1:
            pp = c1.enter_context(tc.tile_pool(name="prow", bufs=1))
            ptf = PT[:].rearrange("p c d -> p (c d)")

            ph = pp.tile([128, PTT * 3], BF16)
            nc.vector.tensor_copy(ph[:], ptf)
            res1 = pp.tile([128, PTT * 3], F32)
            nc.vector.tensor_tensor(res1[:], ptf, ph[:], op=ALU.subtract)
            pm = pp.tile([128, PTT * 3], BF16)
            nc.vector.tensor_copy(pm[:], res1[:])
            res2 = pp.tile([128, PTT * 3], F32)
            nc.vector.tensor_tensor(res2[:], res1[:], pm[:], op=ALU.subtract)
            pl = pp.tile([128, PTT * 3], BF16)
            nc.vector.tensor_copy(pl[:], res2[:])

            sq = pp.tile([128, PTT, 3], F32)
            nc.vector.tensor_tensor(sq[:], PT[:], PT[:], op=ALU.mult)
            p2 = pp.tile([128, PTT], F32)
            nc.vector.tensor_reduce(p2[:], sq[:], axis=mybir.AxisListType.X, op=ALU.add)
            np2 = pp.tile([128, PTT], F32)
            nc.vector.tensor_scalar(np2[:], p2[:], -1.0, None, op0=ALU.mult)
            m2h = pp.tile([128, PTT], BF16)
            nc.vector.tensor_copy(m2h[:], np2[:])
            r1 = pp.tile([128, PTT], F32)
            nc.vector.tensor_tensor(r1[:], np2[:], m2h[:], op=ALU.subtract)
            m2m = pp.tile([128, PTT], BF16)
            nc.vector.tensor_copy(m2m[:], r1[:])
            r2 = pp.tile([128, PTT], F32)
            nc.vector.tensor_tensor(r2[:], r1[:], m2m[:], op=ALU.subtract)
            m2l = pp.tile([128, PTT], BF16)
            nc.vector.tensor_copy(m2l[:], r2[:])

            qf = pp.tile([128, PTT * 3], F32)
            nc.vector.tensor_scalar(qf[:], ptf, KQ, 0.5, op0=ALU.mult, op1=ALU.add)
            qi = pp.tile([128, PTT * 3], I32)
            nc.vector.tensor_copy(qi[:], qf[:])
            qf2 = pp.tile([128, PTT * 3], F32)
            nc.vector.tensor_copy(qf2[:], qi[:])
            qh = pp.tile([128, PTT * 3], BF16)
            nc.vector.tensor_copy(qh[:], qf2[:])
            qres = pp.tile([128, PTT * 3], F32)
            nc.vector.tensor_tensor(qres[:], qf2[:], qh[:], op=ALU.subtract)
            ql = pp.tile([128, PTT * 3], BF16)
            nc.vector.tensor_copy(ql[:], qres[:])

            def row_dma(dst_row, src_tile, coord):
                if coord is None:
                    src = src_tile[:]
                else:
                    src = src_tile[:].rearrange("p (c d) -> p c d", d=3)[:, :, coord]
                nc.sync.dma_start(RHS[dst_row:dst_row + 1, :], src)

            for d in range(3):
                for ri, t_ in enumerate([ph, pm, pl, ph, pm, ph]):
                    row_dma(d * 6 + ri, t_, d)
            for rr in (18, 19, 20):
                for cb in range(N // 2048):
                    nc.sync.dma_start(RHS[rr:rr + 1, cb * 2048:(cb + 1) * 2048], ONESN[:])
            row_dma(21, m2h, None)
            row_dma(22, m2m, None)
            row_dma(23, m2l, None)

            rvi = pp.tile([4, 128], I32)
            nc.gpsimd.iota(rvi[:], pattern=[[-32768, 128]], base=511 * 32768,
                           channel_multiplier=-128 * 32768)
            rvf = pp.tile([4, 128], F32)
            nc.vector.tensor_copy(rvf[:], rvi[:])
            rvh = pp.tile([4, 128], BF16)
            nc.vector.tensor_copy(rvh[:], rvf[:])
            rv1 = pp.tile([4, 128], F32)
            nc.vector.tensor_tensor(rv1[:], rvf[:], rvh[:], op=ALU.subtract)
            rvm = pp.tile([4, 128], BF16)
            nc.vector.tensor_copy(rvm[:], rv1[:])
            rv2 = pp.tile([4, 128], F32)
            nc.vector.tensor_tensor(rv2[:], rv1[:], rvm[:], op=ALU.subtract)
            rvl = pp.tile([4, 128], BF16)
            nc.vector.tensor_copy(rvl[:], rv2[:])
            for lvl, t_ in enumerate((rvh, rvm, rvl)):
                for chk in range(NCH):
                    nc.sync.dma_start(
                        RHS[32 + lvl:33 + lvl, chk * CS:(chk + 1) * CS], t_[:])
            for cb in range(N // 2048):
                nc.sync.dma_start(RHS[35:36, cb * 2048:(cb + 1) * 2048], ONESN[:])
            for d in range(3):
                row_dma(36 + 2 * d, qh, d)
                row_dma(37 + 2 * d, ql, d)

        # ---------- lhsT builder ----------
        def build_lhsT(c3rows, name):
            with ExitStack() as cc:
                pp = cc.enter_context(tc.tile_pool(name=f"lh_{name}", bufs=1))
                ch_ = pp.tile([3, MC], BF16, tag="ch_")
                nc.vector.tensor_copy(ch_[:], c3rows)
                re1 = pp.tile([3, MC], F32, tag="re1")
                nc.vector.tensor_tensor(re1[:], c3rows, ch_[:], op=ALU.subtract)
                cm_ = pp.tile([3, MC], BF16, tag="cm_")
                nc.vector.tensor_copy(cm_[:], re1[:])
                re2 = pp.tile([3, MC], F32, tag="re2")
                nc.vector.tensor_tensor(re2[:], re1[:], cm_[:], op=ALU.subtract)
                cl_ = pp.tile([3, MC], BF16, tag="cl_")
                nc.vector.tensor_copy(cl_[:], re2[:])
                c2h = pp.tile([3, MC], BF16, tag="c2h")
                nc.vector.tensor_scalar(c2h[:], ch_[:], 2.0, None, op0=ALU.mult)
                c2m = pp.tile([3, MC], BF16, tag="c2m")
                nc.vector.tensor_scalar(c2m[:], cm_[:], 2.0, None, op0=ALU.mult)
                c2l = pp.tile([3, MC], BF16, tag="c2l")
                nc.vector.tensor_scalar(c2l[:], cl_[:], 2.0, None, op0=ALU.mult)
                csq = pp.tile([3, MC], F32, tag="csq")
                nc.vector.tensor_tensor(csq[:], c3rows, c3rows, op=ALU.mult)
                cy0 = pp.tile([1, MC], F32, tag="cy0")
                nc.sync.dma_start(cy0[:], csq[1:2, :])
                cz0 = pp.tile([1, MC], F32, tag="cz0")
                nc.sync.dma_start(cz0[:], csq[2:3, :])
                c2r = pp.tile([1, MC], F32, tag="c2r")
                nc.vector.tensor_tensor(c2r[:], csq[0:1, :], cy0[:], op=ALU.add)
                nc.vector.tensor_tensor(c2r[:], c2r[:], cz0[:], op=ALU.add)
                tr = pp.tile([1, MC], F32, tag="tr")
                nc.vector.tensor_scalar(tr[:], c2r[:], -1.0, RAD2, op0=ALU.mult, op1=ALU.add)
                th_ = pp.tile([1, MC], BF16, tag="th_")
                nc.vector.tensor_copy(th_[:], tr[:])
                tr1 = pp.tile([1, MC], F32, tag="tr1")
                nc.vector.tensor_tensor(tr1[:], tr[:], th_[:], op=ALU.subtract)
                tm_ = pp.tile([1, MC], BF16, tag="tm_")
                nc.vector.tensor_copy(tm_[:], tr1[:])
                tr2 = pp.tile([1, MC], F32, tag="tr2")
                nc.vector.tensor_tensor(tr2[:], tr1[:], tm_[:], op=ALU.subtract)
                tl_ = pp.tile([1, MC], BF16, tag="tl_")
                nc.vector.tensor_copy(tl_[:], tr2[:])

                LH = g.tile([24, MC], BF16, tag=f"LH_{name}")
                for d in range(3):
                    for ri, t_ in enumerate([c2h, c2h, c2h, c2m, c2m, c2l]):
                        nc.sync.dma_start(LH[d * 6 + ri:d * 6 + ri + 1, :], t_[d:d + 1, :])
                nc.sync.dma_start(LH[18:19, :], th_[:])
                nc.sync.dma_start(LH[19:20, :], tm_[:])
                nc.sync.dma_start(LH[20:21, :], tl_[:])
                for rr in (21, 22, 23):
                    nc.sync.dma_start(LH[rr:rr + 1, :], ONESN[:, 0:MC])

            return LH

        LH1 = build_lhsT(C3[:], "q1")

        # ---------- selection (one query) ----------
        def run_query(LH, qname, want_out, xc_pool=None):
            out_tiles = []
            with ExitStack() as cs:
                slotp = cs.enter_context(tc.tile_pool(name=f"slot_{qname}", bufs=1))
                SLOT = [[slotp.tile([128, NCH * 16], F32, tag=f"slot{t}_{d}", name=f"slot_{qname}_{t}_{d}")
                         for d in range(3)] for t in range(TC)]

                selstk = ExitStack()
                sp = selstk.enter_context(tc.tile_pool(name=f"sel_{qname}", bufs=2))
                ps = selstk.enter_context(tc.tile_pool(name=f"ps_{qname}", bufs=2, space="PSUM"))
                psg = selstk.enter_context(tc.tile_pool(name=f"psg_{qname}", bufs=2, space="PSUM"))

                for chk in range(NCH):
                    GS_ = []
                    for d in range(3):
                        gp = psg.tile([128, CS], F32, tag=f"g{d}", name=f"gp{d}")
                        nc.tensor.matmul(gp[:], GM[d][32:42, :],
                                         RHS[32:42, chk * CS:(chk + 1) * CS],
                                         start=True, stop=True)
                        gsb = sp.tile([128, CS], F32, tag=f"gsb{d}")
                        nc.scalar.copy(gsb[:], gp[:])
                        GS_.append(gsb)
                    for t in range(TC):
                        s_ps = ps.tile([128, CS], F32, tag="s")
                        nc.tensor.matmul(s_ps[:], LH[:, t * 128:(t + 1) * 128],
                                         RHS[0:24, chk * CS:(chk + 1) * CS],
                                         start=True, stop=True)
                        for d in range(3):
                            fld = sp.tile([128, CS], F32, tag=f"fld{d}")
                            nc.vector.scalar_tensor_tensor(
                                fld[:], s_ps[:], 0.0, GS_[d][:], op0=ALU.is_gt, op1=ALU.mult)
                            o1 = SLOT[t][d][:, chk * 16:chk * 16 + 8]
                            o2 = SLOT[t][d][:, chk * 16 + 8:chk * 16 + 16]
                            nc.vector.max(o1, fld[:])
                            fr = sp.tile([128, CS], F32, tag=f"fr{d}")
                            nc.vector.match_replace(fr[:], o1, fld[:], 0.0)
                            nc.vector.max(o2, fr[:])

                selstk.close()
                dp = cs.enter_context(tc.tile_pool(name=f"dec_{qname}", bufs=1))
                CBASE = dp.tile([128, NCH * 16], I32, tag="cbase")
                nc.gpsimd.iota(CBASE[:], pattern=[[CS, NCH], [0, 16]], base=CS,
                               channel_multiplier=0)
                for t in range(TC):
                    v0 = SLOT[t][0]
                    vi = dp.tile([128, NCH * 16], I32, tag="vi")
                    nc.vector.tensor_copy(vi[:], v0[:])
                    jr = dp.tile([128, NCH * 16], I32, tag="jr")
                    nc.vector.tensor_scalar(jr[:], vi[:], 15, None, op0=ALU.arith_shift_right)
                    jp1i = dp.tile([128, NCH * 16], I32, tag="jp1i")
                    nc.vector.tensor_tensor(jp1i[:], CBASE[:], jr[:], op=ALU.subtract)
                    jp1 = dp.tile([128, NCH * 16], I16, tag="jp1")
                    nc.vector.tensor_copy(jp1[:], jp1i[:])
                    mask = dp.tile([128, NCH * 16], F32, tag="mask")
                    nc.vector.tensor_scalar(mask[:], v0[:], 0.0, None, op0=ALU.is_gt)
                    scan = dp.tile([128, NCH * 16], F32, tag="scan")
                    nc.vector.tensor_tensor_scan(scan[:], mask[:], mask[:], 0.0,
                                                 op0=ALU.add, op1=ALU.bypass)
                    t1_ = dp.tile([128, NCH * 16], F32, tag="t1_")
                    nc.vector.scalar_tensor_tensor(t1_[:], scan[:], -1.0, mask[:],
                                                   op0=ALU.add, op1=ALU.mult)
                    posf = dp.tile([128, NCH * 16], F32, tag="posf")
                    nc.vector.scalar_tensor_tensor(posf[:], mask[:], -1.0, t1_[:],
                                                   op0=ALU.add, op1=ALU.add)
                    pos = dp.tile([128, NCH * 16], I16, tag="pos")
                    nc.vector.tensor_scalar(pos[:], posf[:], 127.0, None, op0=ALU.min)

                    jp1_d = dp.tile([128, 128], I16, tag="jp1d")
                    nc.gpsimd.local_scatter(jp1_d[:], jp1[:], pos[:], channels=128,
                                            num_elems=128, num_idxs=NCH * 16)
                    xcp = xc_pool if xc_pool is not None else dp
                    XC = xcp.tile([128, 3 * 128], F32, tag=f"XC_{qname}_{t}",
                                  name=f"XC_{qname}_{t}")
                    VM = xcp.tile([128, 128], F32, tag=f"VM_{qname}_{t}",
                                  name=f"VM_{qname}_{t}")
                    nc.vector.tensor_scalar(VM[:], jp1_d[:], 0.0, None, op0=ALU.is_gt)
                    for d in range(3):
                        qi_ = dp.tile([128, NCH * 16], I32, tag=f"qi{d}")
                        nc.vector.tensor_copy(qi_[:], SLOT[t][d][:])
                        qm = dp.tile([128, NCH * 16], I32, tag=f"qm{d}")
                        nc.vector.tensor_scalar(qm[:], qi_[:], 32767, None, op0=ALU.bitwise_and)
                        q16 = dp.tile([128, NCH * 16], I16, tag=f"q16{d}")
                        nc.vector.tensor_copy(q16[:], qm[:])
                        qd = dp.tile([128, 128], I16, tag=f"qd{d}")
                        nc.gpsimd.local_scatter(qd[:], q16[:], pos[:], channels=128,
                                                num_elems=128, num_idxs=NCH * 16)
                        dq = dp.tile([128, 128], F32, tag=f"dq{d}")
                        nc.vector.tensor_scalar(dq[:], qd[:], DEQ, -DEQ,
                                                op0=ALU.mult, op1=ALU.add)
                        vmask = dp.tile([128, 128], F32, tag=f"vm{d}")
                        nc.vector.tensor_scalar(vmask[:], qd[:], 0.0, None, op0=ALU.is_gt)
                        nc.vector.tensor_tensor(XC[:, d * 128:(d + 1) * 128], dq[:], vmask[:],
                                                op=ALU.mult)

                    if want_out:
                        ii = dp.tile([128, 128], I32, tag="ii")
                        nc.vector.tensor_scalar(ii[:], jp1_d[:], -1, None, op0=ALU.add)
                        nc.sync.dma_start(o_ix[t * 128:(t + 1) * 128, :], ii[:])
                        for d in range(3):
                            nc.sync.dma_start(o_fc[t * 128:(t + 1) * 128, :, d],
                                              XC[:, d * 128:(d + 1) * 128])
                    out_tiles.append(dict(XC=XC, VM=VM))
            return out_tiles

        xc_stack = ExitStack()
        xcpool = xc_stack.enter_context(tc.tile_pool(name="xcpool", bufs=1))
        q1 = run_query(LH1, "q1", want_out=False, xc_pool=xcpool)

        mlpab = ExitStack()
        xrtp = mlpab.enter_context(tc.tile_pool(name="xrtp", bufs=1))

        # ---------- MLP ----------
        # phase A: center transposes + rel/nn split rows -> XR20 packed tiles
        XRT = []   # (tile, base) per t
        with ExitStack() as ca:
            mp = ca.enter_context(tc.tile_pool(name="mlpa", bufs=2))
            mps = ca.enter_context(tc.tile_pool(name="mlpaps", bufs=2, space="PSUM"))
            xr_tiles = [xrtp.tile([84, 128 * K], BF16, name=f"XRT{i}", tag=f"XRT{i}")
                        for i in range(3)]
            for t in range(TC):
                XRT.append((xr_tiles[t // 3], (t % 3) * 32))

            ONESdiv = g.tile([128, 1], F32)
            nc.vector.memset(ONESdiv[:], 1.0 / 128.0)
            # (XC/VM consumed in this phase; xc_stack closed after)

            for t in range(TC):
                # center coords per partition: CTR [128, 3]
                ct_ps = mps.tile([128, 128], F32, tag="ctps")
                nc.tensor.transpose(ct_ps[:, 0:3], C3[:, t * 128:(t + 1) * 128],
                                    IDENT[0:3, 0:3])
                ctr = mp.tile([128, 3], F32, tag="ctr")
                nc.vector.tensor_copy(ctr[:], ct_ps[:, 0:3])
                XC = q1[t]["XC"]
                VM = q1[t]["VM"]
                REL = mp.tile([128, 3 * 128], F32, tag="REL")
                for d in range(3):
                    nc.vector.scalar_tensor_tensor(
                        REL[:, d * 128:(d + 1) * 128], XC[:, d * 128:(d + 1) * 128],
                        ctr[:, d:d + 1], VM[:], op0=ALU.subtract, op1=ALU.mult)
                relh = mp.tile([128, 3 * 128], BF16, tag="relh")
                nc.vector.tensor_copy(relh[:], REL[:])
                relr = mp.tile([128, 3 * 128], F32, tag="relr")
                nc.vector.tensor_tensor(relr[:], REL[:], relh[:], op=ALU.subtract)
                rell = mp.tile([128, 3 * 128], BF16, tag="rell")
                nc.vector.tensor_copy(rell[:], relr[:])
                nnh = mp.tile([128, 3 * 128], BF16, tag="nnh")
                nc.vector.tensor_copy(nnh[:], XC[:])
                nnr = mp.tile([128, 3 * 128], F32, tag="nnr")
                nc.vector.tensor_tensor(nnr[:], XC[:], nnh[:], op=ALU.subtract)
                nnl = mp.tile([128, 3 * 128], BF16, tag="nnl")
                nc.vector.tensor_copy(nnl[:], nnr[:])

                xt, xb = XRT[t]
                for d in range(3):
                    rh = relh[:, d * 128:(d + 1) * 128]
                    rl = rell[:, d * 128:(d + 1) * 128]
                    nh = nnh[:, d * 128:(d + 1) * 128]
                    nl = nnl[:, d * 128:(d + 1) * 128]
                    for row, srct in ((d, rh), (3 + d, rh), (6 + d, rl),
                                      (9 + d, nh), (12 + d, nh), (15 + d, nl)):
                        nc.sync.dma_start(xt[xb + row:xb + row + 1, :], srct)
                for rr in (18, 19):
                    for cb in range(128 * K // 2048):
                        nc.sync.dma_start(
                            xt[xb + rr:xb + rr + 1, cb * 2048:(cb + 1) * 2048], ONESN[:])

        # phase B: per-block hidden + mean
        H1 = g.tile([128, MC], F32)
        H2 = g.tile([128, MC], F32)
        with ExitStack() as cb:
            mp = cb.enter_context(tc.tile_pool(name="mlpb", bufs=2))
            mps = cb.enter_context(tc.tile_pool(name="mlpbps", bufs=2, space="PSUM"))
            hacc = cb.enter_context(tc.tile_pool(name="haccp", bufs=1, space="PSUM"))
            hc_a = [hacc.tile([128, 512], F32, tag=f"hacc{i}_a", name=f"hacc{i}_a") for i in range(2)]
            hc_b = [hacc.tile([128, 512], F32, tag=f"hacc{i}_b", name=f"hacc{i}_b") for i in range(2)]
            for t in range(TC):
                xt, xb = XRT[t]
                for ml in range(128):
                    m = t * 128 + ml
                    gidx = m // 512
                    col = m % 512
                    a_ps = mps.tile([128, H], F32, tag="aps")
                    nc.tensor.matmul(a_ps[:], xt[xb:xb + 20, ml * K:(ml + 1) * K],
                                     W20[xb:xb + 20, :], start=True, stop=True)
                    relu = mp.tile([128, H], F32, tag="relu")
                    nc.scalar.activation(relu[:], a_ps[:], ACTF.Relu)
                    nc.tensor.matmul(hc_a[gidx][:, col:col + 1], relu[:, 0:128],
                                     ONESdiv[:], start=True, stop=True)
                    nc.tensor.matmul(hc_b[gidx][:, col:col + 1], relu[:, 128:256],
                                     ONESdiv[:], start=True, stop=True)
            nc.vector.tensor_copy(H1[:, 0:512], hc_a[0][:])
            nc.vector.tensor_copy(H2[:, 0:512], hc_b[0][:])
            nc.vector.tensor_copy(H1[:, 512:MC], hc_a[1][:, 0:MC - 512])
            nc.vector.tensor_copy(H2[:, 512:MC], hc_b[1][:, 0:MC - 512])

        mlpab.close()
        xc_stack.close()

        # phase C: w2 + tanh + clamp
        CL3 = g.tile([3, MC], F32)
        with ExitStack() as cc:
            mp = cc.enter_context(tc.tile_pool(name="mlpc", bufs=1))
            mps = cc.enter_context(tc.tile_pool(name="mlpcps", bufs=2, space="PSUM"))
            hh1 = mp.tile([128, MC], BF16, tag="hh1")
            nc.vector.tensor_copy(hh1[:], H1[:])
            hr1 = mp.tile([128, MC], F32, tag="hr1")
            nc.vector.tensor_tensor(hr1[:], H1[:], hh1[:], op=ALU.subtract)
            hl1 = mp.tile([128, MC], BF16, tag="hl1")
            nc.vector.tensor_copy(hl1[:], hr1[:])
            hh2 = mp.tile([128, MC], BF16, tag="hh2")
            nc.vector.tensor_copy(hh2[:], H2[:])
            hr2 = mp.tile([128, MC], F32, tag="hr2")
            nc.vector.tensor_tensor(hr2[:], H2[:], hh2[:], op=ALU.subtract)
            hl2 = mp.tile([128, MC], BF16, tag="hl2")
            nc.vector.tensor_copy(hl2[:], hr2[:])

            OUT3 = mp.tile([3, MC], F32, tag="OUT3")
            for colb in range(2):
                c0_, c1_ = colb * (MC // 2), (colb + 1) * (MC // 2)
                o_ps = mps.tile([3, MC // 2], F32, tag="ops3")
                terms = []
                for chh, (hh, hl) in enumerate(((hh1, hl1), (hh2, hl2))):
                    terms += [(chh * 2 + 0, hh), (chh * 2 + 0, hl), (chh * 2 + 1, hh)]
                for i, (wi, ht) in enumerate(terms):
                    nc.tensor.matmul(o_ps[:], W2S[:, wi, :], ht[:, c0_:c1_],
                                     start=(i == 0), stop=(i == len(terms) - 1))
                nc.vector.tensor_copy(OUT3[:, c0_:c1_], o_ps[:])

            TH = mp.tile([3, MC], F32, tag="TH")
            nc.scalar.activation(TH[:], OUT3[:], ACTF.Tanh)
            nc.vector.scalar_tensor_tensor(CL3[:], TH[:], MARGIN, C3[:],
                                           op0=ALU.mult, op1=ALU.add)
            nc.vector.tensor_scalar(CL3[:], CL3[:], MX[:], MN[:], op0=ALU.min, op1=ALU.max)
            nc.sync.dma_start(o_cl.rearrange("m d -> d m"), CL3[:])

        # ---------- query 2 ----------
        LH2 = build_lhsT(CL3[:], "q2")
        run_query(LH2, "q2", want_out=True)

    nc.compile()
    return nc


_NC_CACHE = {}


def _host_prep(w1, b1, bn_gamma, bn_beta, bn_mean, bn_var, w2):
    t1 = bn_gamma / np.sqrt(bn_var + BN_EPS)
    t2 = bn_beta - bn_mean * t1
    Wp = w1 * t1[:, None]
    bpp = b1 * t1 + t2
    Wrel = Wp[:, 0:3]
    Wnn = Wp[:, 3:6]
    Wrh, Wrl = _split2(Wrel)
    Wnh, Wnl = _split2(Wnn)
    bh, bl = _split2(bpp)
    W20 = np.zeros((20, H), np.float32)
    for d in range(3):
        W20[d] = Wrh[:, d]
        W20[3 + d] = Wrl[:, d]
        W20[6 + d] = Wrh[:, d]
        W20[9 + d] = Wnh[:, d]
        W20[12 + d] = Wnl[:, d]
        W20[15 + d] = Wnh[:, d]
    W20[18] = bh
    W20[19] = bl
    w2h, w2l = _split2(w2)
    W2S = np.zeros((128, 4, 3), np.float32)
    for half in range(2):
        W2S[:, half * 2 + 0, :] = w2h[:, half * 128:(half + 1) * 128].T
        W2S[:, half * 2 + 1, :] = w2l[:, half * 128:(half + 1) * 128].T
    return (W20.astype(_bf), W2S.astype(_bf))


def _gmask():
    gm = np.zeros((3, 42, 128), np.float32)
    for d in range(3):
        gm[d, 32:36] = 1.0
        gm[d, 36 + 2 * d:38 + 2 * d] = 1.0
    return gm.astype(_bf)


def _inputs_for_core(core, points, grid, W20, W2S):
    b, half = core // 2, core % 2
    gpad = np.full((MC, 3), 1.0e6, np.float32)
    gpad[:MH] = grid[half * MH:(half + 1) * MH]
    return dict(points=np.ascontiguousarray(points[b]), grid=gpad,
                w20=W20, w2s=W2S, gmask=_gmask())


def kernel(points, w1, b1, bn_gamma, bn_beta, bn_mean, bn_var, w2):
    points = np.asarray(points, np.float32)
    W20, W2S = _host_prep(
        np.asarray(w1, np.float32), np.asarray(b1, np.float32),
        np.asarray(bn_gamma, np.float32), np.asarray(bn_beta, np.float32),
        np.asarray(bn_mean, np.float32), np.asarray(bn_var, np.float32),
        np.asarray(w2, np.float32))
    lin = np.linspace(0.0, 1.0, GS, dtype=np.float32)
    gx, gy, gz = np.meshgrid(lin, lin, lin, indexing="ij")
    grid = np.stack([gx, gy, gz], -1).reshape(-1, 3).astype(np.float32)

    if "nc" not in _NC_CACHE:
        _NC_CACHE["nc"] = build_kernel()
    nc = _NC_CACHE["nc"]

    in_maps = [_inputs_for_core(c, points, grid, W20, W2S) for c in range(8)]
    res = run_bass_kernel_spmd(nc, in_maps, list(range(8)),
                               trace=bool(_NC_CACHE.get("trace")))
    kernel.last_results = res

    clamped = np.zeros((B, M, 3), np.float32)
    cluster = np.zeros((B, M, K, 3), np.float32)
    idx = np.zeros((B, M, K), np.int32)
    for core in range(8):
        b, half = core // 2, core % 2
        r = res.results[core]
        clamped[b, half * MH:(half + 1) * MH] = r["clamped_o"][:MH]
        cluster[b, half * MH:(half + 1) * MH] = r["cluster_o"][:MH]
        idx[b, half * MH:(half + 1) * MH] = r["idx_o"][:MH]
    return clamped, cluster, idx


# revision 17
# speedup vs baseline: 1.0271x; 1.0271x over previous
"""DeformablePointCluster TRN2 kernel (8 NeuronCores, SPMD, gather-free).

kernel(**inputs) -> (clamped (4,1728,3) f32, final_cluster (4,1728,128,3) f32,
                     idx (4,1728,128) int32)

Sharding: core i -> batch i//2, center half i%2 (864 centers padded to 896 = 7x128).
Selection: s = 9-d2 via K=24 bf16-split matmul (abs err ~1e-4). Coord-carrying score
fields G_d = (511-jloc)*2^15 + round(p_d*32766/30) + 1 masked by (s>0); per 512-chunk
top-8 x 2 rounds -> first-16-by-index hits/chunk; coords decoded from low 15 bits
(quant step 9.2e-4). Compaction: mask-scan + local_scatter. MLP on dequantized coords.
"""
import numpy as np
import ml_dtypes
from contextlib import ExitStack

import concourse.bass as bass
import concourse.tile as tile
from concourse import bacc, mybir
from concourse.bass_utils import run_bass_kernel_spmd

dt = mybir.dt
F32, BF16, I16, I32 = dt.float32, dt.bfloat16, dt.int16, dt.int32
ALU = mybir.AluOpType
ACTF = mybir.ActivationFunctionType

B, N, GS, K, H = 4, 16384, 12, 128, 256
M = GS * GS * GS
MH = M // 2                 # 864
MC = 896                    # padded (7 tiles)
TC = MC // 128
CS = 512
NCH = N // CS
RAD2 = 9.0
MARGIN = 4.0
BN_EPS = 1e-5
KQ = 32766.0 / 30.0
DEQ = 30.0 / 32766.0
PTT = 128

_bf = ml_dtypes.bfloat16


def _split2(x):
    h = x.astype(_bf).astype(np.float32)
    l = (x - h).astype(_bf).astype(np.float32)
    return h, l


def build_kernel():
    nc = bacc.Bacc("TRN2", target_bir_lowering=False, debug=False, num_devices=8)

    i_pts = nc.dram_tensor("points", [N, 3], F32, kind="ExternalInput").ap()
    i_grid = nc.dram_tensor("grid", [MC, 3], F32, kind="ExternalInput").ap()
    i_w20 = nc.dram_tensor("w20", [20, H], BF16, kind="ExternalInput").ap()
    i_w2s = nc.dram_tensor("w2s", [128, 4, 3], BF16, kind="ExternalInput").ap()
    i_gm = nc.dram_tensor("gmask", [3, 42, 128], BF16, kind="ExternalInput").ap()

    o_cl = nc.dram_tensor("clamped_o", [MC, 3], F32, kind="ExternalOutput").ap()
    o_fc = nc.dram_tensor("cluster_o", [MC, K, 3], F32, kind="ExternalOutput").ap()
    o_ix = nc.dram_tensor("idx_o", [MC, K], I32, kind="ExternalOutput").ap()

    with tile.TileContext(nc) as tc, ExitStack() as ctx:
        g = ctx.enter_context(tc.tile_pool(name="glob", bufs=1))

        PT = g.tile([128, PTT, 3], F32)
        nc.sync.dma_start(PT[:], i_pts.rearrange("(p c) d -> p c d", p=128))
        G3 = g.tile([3, MC], F32)
        nc.sync.dma_start(G3[:], i_grid.rearrange("m d -> d m"))
        W20 = g.tile([84, H], BF16)
        for bb in (0, 32, 64):
            nc.sync.dma_start(W20[bb:bb + 20, :], i_w20)
        W2S = g.tile([128, 4, 3], BF16)
        nc.sync.dma_start(W2S[:], i_w2s)

        ONESN = g.tile([1, 2048], BF16)
        nc.vector.memset(ONESN[:], 1.0)
        GM = []
        for d in range(3):
            gm = g.tile([42, 128], BF16, name=f"GM{d}", tag=f"GM{d}")
            nc.sync.dma_start(gm[:], i_gm[d])
            GM.append(gm)
        ONESF = g.tile([128, 128], F32)
        nc.vector.memset(ONESF[:], 1.0)
        IDENT = g.tile([128, 128], F32)
        nc.gpsimd.affine_select(IDENT[:], ONESF[:], pattern=[[-1, 128]],
                                compare_op=ALU.is_equal, fill=0.0,
                                base=0, channel_multiplier=1)

        # ---------- min/max ----------
        MN = g.tile([3, 1], F32)
        MX = g.tile([3, 1], F32)
        with ExitStack() as c0:
            pp = c0.enter_context(tc.tile_pool(name="prep0", bufs=1))
            pps = c0.enter_context(tc.tile_pool(name="prep0ps", bufs=1, space="PSUM"))
            mn_p = pp.tile([128, 3], F32)
            mx_p = pp.tile([128, 3], F32)
            ptT = PT[:].rearrange("p c d -> p d c")
            nc.vector.tensor_reduce(mn_p[:], ptT, axis=mybir.AxisListType.X, op=ALU.min)
            nc.vector.tensor_reduce(mx_p[:], ptT, axis=mybir.AxisListType.X, op=ALU.max)
            mnT = pps.tile([128, 128], F32, tag="mnT")
            mxT = pps.tile([128, 128], F32, tag="mxT")
            nc.tensor.transpose(mnT[0:3, :], mn_p[:, 0:3], IDENT[:])
            nc.tensor.transpose(mxT[0:3, :], mx_p[:, 0:3], IDENT[:])
            nc.vector.tensor_reduce(MN[:], mnT[0:3, :], axis=mybir.AxisListType.X, op=ALU.min)
            nc.vector.tensor_reduce(MX[:], mxT[0:3, :], axis=mybir.AxisListType.X, op=ALU.max)

        SCL = g.tile([3, 1], F32)
        nc.vector.scalar_tensor_tensor(SCL[:], MX[:], -8.0, MN[:], op0=ALU.add, op1=ALU.subtract)
        MN4 = g.tile([3, 1], F32)
        nc.vector.tensor_scalar(MN4[:], MN[:], 4.0, None, op0=ALU.add)
        C3 = g.tile([3, MC], F32)
        nc.vector.tensor_scalar(C3[:], G3[:], SCL[:], MN4[:], op0=ALU.mult, op1=ALU.add)

        # ---------- point-side RHS rows; matmul operand base must be 0/32/64 ----------
        # rows 0-23 d2; 32-34 ramp splits; 35 ones; 36-41 q splits (x,y,z)
        RHS = g.tile([42, N], BF16)
        with ExitStack() as c1:
            pp = c1.enter_context(tc.tile_pool(name="prow", bufs=1))
            ptf = PT[:].rearrange("p c d -> p (c d)")

            ph = pp.tile([128, PTT * 3], BF16)
            nc.vector.tensor_copy(ph[:], ptf)
            res1 = pp.tile([128, PTT * 3], F32)
            nc.vector.tensor_tensor(res1[:], ptf, ph[:], op=ALU.subtract)
            pm = pp.tile([128, PTT * 3], BF16)
            nc.vector.tensor_copy(pm[:], res1[:])
            res2 = pp.tile([128, PTT * 3], F32)
            nc.vector.tensor_tensor(res2[:], res1[:], pm[:], op=ALU.subtract)
            pl = pp.tile([128, PTT * 3], BF16)
            nc.vector.tensor_copy(pl[:], res2[:])

            sq = pp.tile([128, PTT, 3], F32)
            nc.vector.tensor_tensor(sq[:], PT[:], PT[:], op=ALU.mult)
            p2 = pp.tile([128, PTT], F32)
            nc.vector.tensor_reduce(p2[:], sq[:], axis=mybir.AxisListType.X, op=ALU.add)
            np2 = pp.tile([128, PTT], F32)
            nc.vector.tensor_scalar(np2[:], p2[:], -1.0, None, op0=ALU.mult)
            m2h = pp.tile([128, PTT], BF16)
            nc.vector.tensor_copy(m2h[:], np2[:])
            r1 = pp.tile([128, PTT], F32)
            nc.vector.tensor_tensor(r1[:], np2[:], m2h[:], op=ALU.subtract)
            m2m = pp.tile([128, PTT], BF16)
            nc.vector.tensor_copy(m2m[:], r1[:])
            r2 = pp.tile([128, PTT], F32)
            nc.vector.tensor_tensor(r2[:], r1[:], m2m[:], op=ALU.subtract)
            m2l = pp.tile([128, PTT], BF16)
            nc.vector.tensor_copy(m2l[:], r2[:])

            qf = pp.tile([128, PTT * 3], F32)
            nc.vector.tensor_scalar(qf[:], ptf, KQ, 0.5, op0=ALU.mult, op1=ALU.add)
            qi = pp.tile([128, PTT * 3], I32)
            nc.vector.tensor_copy(qi[:], qf[:])
            qf2 = pp.tile([128, PTT * 3], F32)
            nc.vector.tensor_copy(qf2[:], qi[:])
            qh = pp.tile([128, PTT * 3], BF16)
            nc.vector.tensor_copy(qh[:], qf2[:])
            qres = pp.tile([128, PTT * 3], F32)
            nc.vector.tensor_tensor(qres[:], qf2[:], qh[:], op=ALU.subtract)
            ql = pp.tile([128, PTT * 3], BF16)
            nc.vector.tensor_copy(ql[:], qres[:])

            def row_dma(dst_row, src_tile, coord):
                if coord is None:
                    src = src_tile[:]
                else:
                    src = src_tile[:].rearrange("p (c d) -> p c d", d=3)[:, :, coord]
                nc.sync.dma_start(RHS[dst_row:dst_row + 1, :], src)

            for d in range(3):
                for ri, t_ in enumerate([ph, pm, pl, ph, pm, ph]):
                    row_dma(d * 6 + ri, t_, d)
            for rr in (18, 19, 20):
                for cb in range(N // 2048):
                    nc.sync.dma_start(RHS[rr:rr + 1, cb * 2048:(cb + 1) * 2048], ONESN[:])
            row_dma(21, m2h, None)
            row_dma(22, m2m, None)
            row_dma(23, m2l, None)

            rvi = pp.tile([4, 128], I32)
            nc.gpsimd.iota(rvi[:], pattern=[[-32768, 128]], base=511 * 32768,
                           channel_multiplier=-128 * 32768)
            rvf = pp.tile([4, 128], F32)
            nc.vector.tensor_copy(rvf[:], rvi[:])
            rvh = pp.tile([4, 128], BF16)
            nc.vector.tensor_copy(rvh[:], rvf[:])
            rv1 = pp.tile([4, 128], F32)
            nc.vector.tensor_tensor(rv1[:], rvf[:], rvh[:], op=ALU.subtract)
            rvm = pp.tile([4, 128], BF16)
            nc.vector.tensor_copy(rvm[:], rv1[:])
            rv2 = pp.tile([4, 128], F32)
            nc.vector.tensor_tensor(rv2[:], rv1[:], rvm[:], op=ALU.subtract)
            rvl = pp.tile([4, 128], BF16)
            nc.vector.tensor_copy(rvl[:], rv2[:])
            for lvl, t_ in enumerate((rvh, rvm, rvl)):
                for chk in range(NCH):
                    nc.sync.dma_start(
                        RHS[32 + lvl:33 + lvl, chk * CS:(chk + 1) * CS], t_[:])
            for cb in range(N // 2048):
                nc.sync.dma_start(RHS[35:36, cb * 2048:(cb + 1) * 2048], ONESN[:])
            for d in range(3):
                row_dma(36 + 2 * d, qh, d)
                row_dma(37 + 2 * d, ql, d)

        # ---------- lhsT builder ----------
        def build_lhsT(c3rows, name):
            with ExitStack() as cc:
                pp = cc.enter_context(tc.tile_pool(name=f"lh_{name}", bufs=1))
                ch_ = pp.tile([3, MC], BF16, tag="ch_")
                nc.vector.tensor_copy(ch_[:], c3rows)
                re1 = pp.tile([3, MC], F32, tag="re1")
                nc.vector.tensor_tensor(re1[:], c3rows, ch_[:], op=ALU.subtract)
                cm_ = pp.tile([3, MC], BF16, tag="cm_")
                nc.vector.tensor_copy(cm_[:], re1[:])
                re2 = pp.tile([3, MC], F32, tag="re2")
                nc.vector.tensor_tensor(re2[:], re1[:], cm_[:], op=ALU.subtract)
                cl_ = pp.tile([3, MC], BF16, tag="cl_")
                nc.vector.tensor_copy(cl_[:], re2[:])
                c2h = pp.tile([3, MC], BF16, tag="c2h")
                nc.vector.tensor_scalar(c2h[:], ch_[:], 2.0, None, op0=ALU.mult)
                c2m = pp.tile([3, MC], BF16, tag="c2m")
                nc.vector.tensor_scalar(c2m[:], cm_[:], 2.0, None, op0=ALU.mult)
                c2l = pp.tile([3, MC], BF16, tag="c2l")
                nc.vector.tensor_scalar(c2l[:], cl_[:], 2.0, None, op0=ALU.mult)
                csq = pp.tile([3, MC], F32, tag="csq")
                nc.vector.tensor_tensor(csq[:], c3rows, c3rows, op=ALU.mult)
                cy0 = pp.tile([1, MC], F32, tag="cy0")
                nc.sync.dma_start(cy0[:], csq[1:2, :])
                cz0 = pp.tile([1, MC], F32, tag="cz0")
                nc.sync.dma_start(cz0[:], csq[2:3, :])
                c2r = pp.tile([1, MC], F32, tag="c2r")
                nc.vector.tensor_tensor(c2r[:], csq[0:1, :], cy0[:], op=ALU.add)
                nc.vector.tensor_tensor(c2r[:], c2r[:], cz0[:], op=ALU.add)
                tr = pp.tile([1, MC], F32, tag="tr")
                nc.vector.tensor_scalar(tr[:], c2r[:], -1.0, RAD2, op0=ALU.mult, op1=ALU.add)
                th_ = pp.tile([1, MC], BF16, tag="th_")
                nc.vector.tensor_copy(th_[:], tr[:])
                tr1 = pp.tile([1, MC], F32, tag="tr1")
                nc.vector.tensor_tensor(tr1[:], tr[:], th_[:], op=ALU.subtract)
                tm_ = pp.tile([1, MC], BF16, tag="tm_")
                nc.vector.tensor_copy(tm_[:], tr1[:])
                tr2 = pp.tile([1, MC], F32, tag="tr2")
                nc.vector.tensor_tensor(tr2[:], tr1[:], tm_[:], op=ALU.subtract)
                tl_ = pp.tile([1, MC], BF16, tag="tl_")
                nc.vector.tensor_copy(tl_[:], tr2[:])

                LH = g.tile([24, MC], BF16, tag=f"LH_{name}")
                for d in range(3):
                    for ri, t_ in enumerate([c2h, c2h, c2h, c2m, c2m, c2l]):
                        nc.sync.dma_start(LH[d * 6 + ri:d * 6 + ri + 1, :], t_[d:d + 1, :])
                nc.sync.dma_start(LH[18:19, :], th_[:])
                nc.sync.dma_start(LH[19:20, :], tm_[:])
                nc.sync.dma_start(LH[20:21, :], tl_[:])
                for rr in (21, 22, 23):
                    nc.sync.dma_start(LH[rr:rr + 1, :], ONESN[:, 0:MC])

            return LH

        LH1 = build_lhsT(C3[:], "q1")

        # ---------- selection (one query) ----------
        def run_query(LH, qname, want_out, xc_pool=None):
            out_tiles = []
            with ExitStack() as cs:
                slotp = cs.enter_context(tc.tile_pool(name=f"slot_{qname}", bufs=1))
                SLOT = [[slotp.tile([128, NCH * 16], F32, tag=f"slot{t}_{d}", name=f"slot_{qname}_{t}_{d}")
                         for d in range(3)] for t in range(TC)]

                selstk = ExitStack()
                sp = selstk.enter_context(tc.tile_pool(name=f"sel_{qname}", bufs=2))
                ps = selstk.enter_context(tc.tile_pool(name=f"ps_{qname}", bufs=2, space="PSUM"))
                psg = selstk.enter_context(tc.tile_pool(name=f"psg_{qname}", bufs=2, space="PSUM"))

                for chk in range(NCH):
                    GS_ = []
                    for d in range(3):
                        gp = psg.tile([128, CS], F32, tag=f"g{d}", name=f"gp{d}")
                        nc.tensor.matmul(gp[:], GM[d][32:42, :],
                                         RHS[32:42, chk * CS:(chk + 1) * CS],
                                         start=True, stop=True)
                        gsb = sp.tile([128, CS], F32, tag=f"gsb{d}")
                        nc.scalar.copy(gsb[:], gp[:])
                        GS_.append(gsb)
                    for t in range(TC):
                        s_ps = ps.tile([128, CS], F32, tag="s")
                        nc.tensor.matmul(s_ps[:], LH[:, t * 128:(t + 1) * 128],
                                         RHS[0:24, chk * CS:(chk + 1) * CS],
                                         start=True, stop=True)
                        for d in range(3):
                            fld = sp.tile([128, CS], F32, tag=f"fld{d}")
                            nc.vector.scalar_tensor_tensor(
                                fld[:], s_ps[:], 0.0, GS_[d][:], op0=ALU.is_gt, op1=ALU.mult)
                            o1 = SLOT[t][d][:, chk * 16:chk * 16 + 8]
                            o2 = SLOT[t][d][:, chk * 16 + 8:chk * 16 + 16]
                            nc.vector.max(o1, fld[:])
                            fr = sp.tile([128, CS], F32, tag=f"fr{d}")
                            nc.vector.match_replace(fr[:], o1, fld[:], 0.0)
                            nc.vector.max(o2, fr[:])

                selstk.close()
                dp = cs.enter_context(tc.tile_pool(name=f"dec_{qname}", bufs=1))
                CBASE = dp.tile([128, NCH * 16], I32, tag="cbase")
                nc.gpsimd.iota(CBASE[:], pattern=[[CS, NCH], [0, 16]], base=CS,
                               channel_multiplier=0)
                for t in range(TC):
                    v0 = SLOT[t][0]
                    vi = dp.tile([128, NCH * 16], I32, tag="vi")
                    nc.vector.tensor_copy(vi[:], v0[:])
                    jr = dp.tile([128, NCH * 16], I32, tag="jr")
                    nc.vector.tensor_scalar(jr[:], vi[:], 15, None, op0=ALU.arith_shift_right)
                    jp1i = dp.tile([128, NCH * 16], I32, tag="jp1i")
                    nc.vector.tensor_tensor(jp1i[:], CBASE[:], jr[:], op=ALU.subtract)
                    jp1 = dp.tile([128, NCH * 16], I16, tag="jp1")
                    nc.vector.tensor_copy(jp1[:], jp1i[:])
                    mask = dp.tile([128, NCH * 16], F32, tag="mask")
                    nc.vector.tensor_scalar(mask[:], v0[:], 0.0, None, op0=ALU.is_gt)
                    scan = dp.tile([128, NCH * 16], F32, tag="scan")
                    nc.vector.tensor_tensor_scan(scan[:], mask[:], mask[:], 0.0,
                                                 op0=ALU.add, op1=ALU.bypass)
                    t1_ = dp.tile([128, NCH * 16], F32, tag="t1_")
                    nc.vector.scalar_tensor_tensor(t1_[:], scan[:], -1.0, mask[:],
                                                   op0=ALU.add, op1=ALU.mult)
                    posf = dp.tile([128, NCH * 16], F32, tag="posf")
                    nc.vector.scalar_tensor_tensor(posf[:], mask[:], -1.0, t1_[:],
                                                   op0=ALU.add, op1=ALU.add)
                    pos = dp.tile([128, NCH * 16], I16, tag="pos")
                    nc.vector.tensor_scalar(pos[:], posf[:], 127.0, None, op0=ALU.min)

                    jp1_d = dp.tile([128, 128], I16, tag="jp1d")
                    nc.gpsimd.local_scatter(jp1_d[:], jp1[:], pos[:], channels=128,
                                            num_elems=128, num_idxs=NCH * 16)
                    xcp = xc_pool if xc_pool is not None else dp
                    XC = xcp.tile([128, 3 * 128], F32, tag=f"XC_{qname}_{t}",
                                  name=f"XC_{qname}_{t}")
                    VM = xcp.tile([128, 128], F32, tag=f"VM_{qname}_{t}",
                                  name=f"VM_{qname}_{t}")
                    nc.vector.tensor_scalar(VM[:], jp1_d[:], 0.0, None, op0=ALU.is_gt)
                    for d in range(3):
                        qi_ = dp.tile([128, NCH * 16], I32, tag=f"qi{d}")
                        nc.vector.tensor_copy(qi_[:], SLOT[t][d][:])
                        qm = dp.tile([128, NCH * 16], I32, tag=f"qm{d}")
                        nc.vector.tensor_scalar(qm[:], qi_[:], 32767, None, op0=ALU.bitwise_and)
                        q16 = dp.tile([128, NCH * 16], I16, tag=f"q16{d}")
                        nc.vector.tensor_copy(q16[:], qm[:])
                        qd = dp.tile([128, 128], I16, tag=f"qd{d}")
                        nc.gpsimd.local_scatter(qd[:], q16[:], pos[:], channels=128,
                                                num_elems=128, num_idxs=NCH * 16)
                        dq = dp.tile([128, 128], F32, tag=f"dq{d}")
                        nc.vector.tensor_scalar(dq[:], qd[:], DEQ, -DEQ,
                                                op0=ALU.mult, op1=ALU.add)
                        vmask = dp.tile([128, 128], F32, tag=f"vm{d}")
                        nc.vector.tensor_scalar(vmask[:], qd[:], 0.0, None, op0=ALU.is_gt)
                        nc.vector.tensor_tensor(XC[:, d * 128:(d + 1) * 128], dq[:], vmask[:],
                                                op=ALU.mult)

                    if want_out:
                        ii = dp.tile([128, 128], I32, tag="ii")
                        nc.vector.tensor_scalar(ii[:], jp1_d[:], -1, None, op0=ALU.add)
                        nc.sync.dma_start(o_ix[t * 128:(t + 1) * 128, :], ii[:])
                        for d in range(3):
                            nc.sync.dma_start(o_fc[t * 128:(t + 1) * 128, :, d],
                                              XC[:, d * 128:(d + 1) * 128])
                    out_tiles.append(dict(XC=XC, VM=VM))
            return out_tiles

        xc_stack = ExitStack()
        xcpool = xc_stack.enter_context(tc.tile_pool(name="xcpool", bufs=1))
        with nc.named_scope("sel_q1"):
            q1 = run_query(LH1, "q1", want_out=False, xc_pool=xcpool)

        mlpab = ExitStack()
        xrtp = mlpab.enter_context(tc.tile_pool(name="xrtp", bufs=1))

        # ---------- MLP ----------
        # phase A: center transposes + rel/nn split rows -> XR20 packed tiles
        XRT = []   # (tile, base) per t
        with nc.named_scope("mlp_a"), ExitStack() as ca:
            mp = ca.enter_context(tc.tile_pool(name="mlpa", bufs=2))
            mps = ca.enter_context(tc.tile_pool(name="mlpaps", bufs=2, space="PSUM"))
            xr_tiles = [xrtp.tile([84, 128 * K], BF16, name=f"XRT{i}", tag=f"XRT{i}")
                        for i in range(3)]
            for t in range(TC):
                XRT.append((xr_tiles[t // 3], (t % 3) * 32))

            ONESdiv = g.tile([128, 1], F32)
            nc.vector.memset(ONESdiv[:], 1.0 / 128.0)
            # (XC/VM consumed in this phase; xc_stack closed after)

            for t in range(TC):
                # center coords per partition: CTR [128, 3]
                ct_ps = mps.tile([128, 128], F32, tag="ctps")
                nc.tensor.transpose(ct_ps[:, 0:3], C3[:, t * 128:(t + 1) * 128],
                                    IDENT[0:3, 0:3])
                ctr = mp.tile([128, 3], F32, tag="ctr")
                nc.vector.tensor_copy(ctr[:], ct_ps[:, 0:3])
                XC = q1[t]["XC"]
                VM = q1[t]["VM"]
                REL = mp.tile([128, 3 * 128], F32, tag="REL")
                for d in range(3):
                    nc.vector.scalar_tensor_tensor(
                        REL[:, d * 128:(d + 1) * 128], XC[:, d * 128:(d + 1) * 128],
                        ctr[:, d:d + 1], VM[:], op0=ALU.subtract, op1=ALU.mult)
                relh = mp.tile([128, 3 * 128], BF16, tag="relh")
                nc.vector.tensor_copy(relh[:], REL[:])
                relr = mp.tile([128, 3 * 128], F32, tag="relr")
                nc.vector.tensor_tensor(relr[:], REL[:], relh[:], op=ALU.subtract)
                rell = mp.tile([128, 3 * 128], BF16, tag="rell")
                nc.vector.tensor_copy(rell[:], relr[:])
                nnh = mp.tile([128, 3 * 128], BF16, tag="nnh")
                nc.vector.tensor_copy(nnh[:], XC[:])
                nnr = mp.tile([128, 3 * 128], F32, tag="nnr")
                nc.vector.tensor_tensor(nnr[:], XC[:], nnh[:], op=ALU.subtract)
                nnl = mp.tile([128, 3 * 128], BF16, tag="nnl")
                nc.vector.tensor_copy(nnl[:], nnr[:])

                xt, xb = XRT[t]
                for d in range(3):
                    rh = relh[:, d * 128:(d + 1) * 128]
                    rl = rell[:, d * 128:(d + 1) * 128]
                    nh = nnh[:, d * 128:(d + 1) * 128]
                    nl = nnl[:, d * 128:(d + 1) * 128]
                    for row, srct in ((d, rh), (3 + d, rh), (6 + d, rl),
                                      (9 + d, nh), (12 + d, nh), (15 + d, nl)):
                        nc.sync.dma_start(xt[xb + row:xb + row + 1, :], srct)
                for rr in (18, 19):
                    for cb in range(128 * K // 2048):
                        nc.sync.dma_start(
                            xt[xb + rr:xb + rr + 1, cb * 2048:(cb + 1) * 2048], ONESN[:])

        # phase B: per-block hidden + mean
        H1 = g.tile([128, MC], F32)
        H2 = g.tile([128, MC], F32)
        with nc.named_scope("mlp_b"), ExitStack() as cb:
            mp = cb.enter_context(tc.tile_pool(name="mlpb", bufs=2))
            mps = cb.enter_context(tc.tile_pool(name="mlpbps", bufs=2, space="PSUM"))
            hacc = cb.enter_context(tc.tile_pool(name="haccp", bufs=1, space="PSUM"))
            hc_a = [hacc.tile([128, 512], F32, tag=f"hacc{i}_a", name=f"hacc{i}_a") for i in range(2)]
            hc_b = [hacc.tile([128, 512], F32, tag=f"hacc{i}_b", name=f"hacc{i}_b") for i in range(2)]
            for t in range(TC):
                xt, xb = XRT[t]
                for ml in range(128):
                    m = t * 128 + ml
                    gidx = m // 512
                    col = m % 512
                    a_ps = mps.tile([128, H], F32, tag="aps")
                    nc.tensor.matmul(a_ps[:], xt[xb:xb + 20, ml * K:(ml + 1) * K],
                                     W20[xb:xb + 20, :], start=True, stop=True)
                    relu = mp.tile([128, H], F32, tag="relu")
                    nc.scalar.activation(relu[:], a_ps[:], ACTF.Relu)
                    nc.tensor.matmul(hc_a[gidx][:, col:col + 1], relu[:, 0:128],
                                     ONESdiv[:], start=True, stop=True)
                    nc.tensor.matmul(hc_b[gidx][:, col:col + 1], relu[:, 128:256],
                                     ONESdiv[:], start=True, stop=True)
            nc.vector.tensor_copy(H1[:, 0:512], hc_a[0][:])
            nc.vector.tensor_copy(H2[:, 0:512], hc_b[0][:])
            nc.vector.tensor_copy(H1[:, 512:MC], hc_a[1][:, 0:MC - 512])
            nc.vector.tensor_copy(H2[:, 512:MC], hc_b[1][:, 0:MC - 512])

        mlpab.close()
        xc_stack.close()

        # phase C: w2 + tanh + clamp
        CL3 = g.tile([3, MC], F32)
        with nc.named_scope("mlp_c"), ExitStack() as cc:
            mp = cc.enter_context(tc.tile_pool(name="mlpc", bufs=1))
            mps = cc.enter_context(tc.tile_pool(name="mlpcps", bufs=2, space="PSUM"))
            hh1 = mp.tile([128, MC], BF16, tag="hh1")
            nc.vector.tensor_copy(hh1[:], H1[:])
            hr1 = mp.tile([128, MC], F32, tag="hr1")
            nc.vector.tensor_tensor(hr1[:], H1[:], hh1[:], op=ALU.subtract)
            hl1 = mp.tile([128, MC], BF16, tag="hl1")
            nc.vector.tensor_copy(hl1[:], hr1[:])
            hh2 = mp.tile([128, MC], BF16, tag="hh2")
            nc.vector.tensor_copy(hh2[:], H2[:])
            hr2 = mp.tile([128, MC], F32, tag="hr2")
            nc.vector.tensor_tensor(hr2[:], H2[:], hh2[:], op=ALU.subtract)
            hl2 = mp.tile([128, MC], BF16, tag="hl2")
            nc.vector.tensor_copy(hl2[:], hr2[:])

            OUT3 = mp.tile([3, MC], F32, tag="OUT3")
            for colb in range(2):
                c0_, c1_ = colb * (MC // 2), (colb + 1) * (MC // 2)
                o_ps = mps.tile([3, MC // 2], F32, tag="ops3")
                terms = []
                for chh, (hh, hl) in enumerate(((hh1, hl1), (hh2, hl2))):
                    terms += [(chh * 2 + 0, hh), (chh * 2 + 0, hl), (chh * 2 + 1, hh)]
                for i, (wi, ht) in enumerate(terms):
                    nc.tensor.matmul(o_ps[:], W2S[:, wi, :], ht[:, c0_:c1_],
                                     start=(i == 0), stop=(i == len(terms) - 1))
                nc.vector.tensor_copy(OUT3[:, c0_:c1_], o_ps[:])

            TH = mp.tile([3, MC], F32, tag="TH")
            nc.scalar.activation(TH[:], OUT3[:], ACTF.Tanh)
            nc.vector.scalar_tensor_tensor(CL3[:], TH[:], MARGIN, C3[:],
                                           op0=ALU.mult, op1=ALU.add)
            nc.vector.tensor_scalar(CL3[:], CL3[:], MX[:], MN[:], op0=ALU.min, op1=ALU.max)
            nc.sync.dma_start(o_cl.rearrange("m d -> d m"), CL3[:])

        # ---------- query 2 ----------
        LH2 = build_lhsT(CL3[:], "q2")
        with nc.named_scope("sel_q2"):
            run_query(LH2, "q2", want_out=True)

    nc.compile()
    return nc


_NC_CACHE = {}


def _host_prep(w1, b1, bn_gamma, bn_beta, bn_mean, bn_var, w2):
    t1 = bn_gamma / np.sqrt(bn_var + BN_EPS)
    t2 = bn_beta - bn_mean * t1
    Wp = w1 * t1[:, None]
    bpp = b1 * t1 + t2
    Wrel = Wp[:, 0:3]
    Wnn = Wp[:, 3:6]
    Wrh, Wrl = _split2(Wrel)
    Wnh, Wnl = _split2(Wnn)
    bh, bl = _split2(bpp)
    W20 = np.zeros((20, H), np.float32)
    for d in range(3):
        W20[d] = Wrh[:, d]
        W20[3 + d] = Wrl[:, d]
        W20[6 + d] = Wrh[:, d]
        W20[9 + d] = Wnh[:, d]
        W20[12 + d] = Wnl[:, d]
        W20[15 + d] = Wnh[:, d]
    W20[18] = bh
    W20[19] = bl
    w2h, w2l = _split2(w2)
    W2S = np.zeros((128, 4, 3), np.float32)
    for half in range(2):
        W2S[:, half * 2 + 0, :] = w2h[:, half * 128:(half + 1) * 128].T
        W2S[:, half * 2 + 1, :] = w2l[:, half * 128:(half + 1) * 128].T
    return (W20.astype(_bf), W2S.astype(_bf))


def _gmask():
    gm = np.zeros((3, 42, 128), np.float32)
    for d in range(3):
        gm[d, 32:36] = 1.0
        gm[d, 36 + 2 * d:38 + 2 * d] = 1.0
    return gm.astype(_bf)


def _inputs_for_core(core, points, grid, W20, W2S):
    b, half = core // 2, core % 2
    gpad = np.full((MC, 3), 1.0e6, np.float32)
    gpad[:MH] = grid[half * MH:(half + 1) * MH]
    return dict(points=np.ascontiguousarray(points[b]), grid=gpad,
                w20=W20, w2s=W2S, gmask=_gmask())


def kernel(points, w1, b1, bn_gamma, bn_beta, bn_mean, bn_var, w2):
    points = np.asarray(points, np.float32)
    W20, W2S = _host_prep(
        np.asarray(w1, np.float32), np.asarray(b1, np.float32),
        np.asarray(bn_gamma, np.float32), np.asarray(bn_beta, np.float32),
        np.asarray(bn_mean, np.float32), np.asarray(bn_var, np.float32),
        np.asarray(w2, np.float32))
    lin = np.linspace(0.0, 1.0, GS, dtype=np.float32)
    gx, gy, gz = np.meshgrid(lin, lin, lin, indexing="ij")
    grid = np.stack([gx, gy, gz], -1).reshape(-1, 3).astype(np.float32)

    if "nc" not in _NC_CACHE:
        _NC_CACHE["nc"] = build_kernel()
    nc = _NC_CACHE["nc"]

    in_maps = [_inputs_for_core(c, points, grid, W20, W2S) for c in range(8)]
    res = run_bass_kernel_spmd(nc, in_maps, list(range(8)),
                               trace=bool(_NC_CACHE.get("trace")))
    kernel.last_results = res

    clamped = np.zeros((B, M, 3), np.float32)
    cluster = np.zeros((B, M, K, 3), np.float32)
    idx = np.zeros((B, M, K), np.int32)
    for core in range(8):
        b, half = core // 2, core % 2
        r = res.results[core]
        clamped[b, half * MH:(half + 1) * MH] = r["clamped_o"][:MH]
        cluster[b, half * MH:(half + 1) * MH] = r["cluster_o"][:MH]
        idx[b, half * MH:(half + 1) * MH] = r["idx_o"][:MH]
    return clamped, cluster, idx
